# revision 41
# baseline (speedup 1.0000x reference)
"""Multi-head attention (B=2, T=2048, E=1024, H=16) on 8 TRN2 NeuronCores.

Sharding: core c handles batch c//4 and head group c%4 (4 heads of 64 dims
-> 256 columns of w_Q/w_K/w_V and of the output). Pure SPMD, no collectives:
every core runs the same NEFF on its own input shard.

Per-core kernel (all matmul operands bf16, PSUM/softmax math fp32):
  xT [E, T] (host pre-transposed), wq/wk/wv [E, 256]
  1. QT/KT per head-pair p: [128, T] = (w pair-slice)^T @ xT   (PE)
  2. V per s-tile: [128, 4*65] with a ones column per head     (PE + DVE copy)
  3. scores transposed per head: ST[s, t] = K Q^T, two heads packed into
     PE row groups (K=64 each) writing one [128, 1024] PSUM tile
  4. exp via ACT straight from PSUM, scale=1/8 folded into the activation
     affine, bf16 out -> PT
  5. attn V-stationary: acc[d,t] = V_aug^T @ PT at N=512 per (head,s-tile),
     accumulated over 16 s-chunks; row 64 = softmax denominator (ones col)
  6. acc -> SBUF copy -> DMA out un-normalized; the host divides by row 64
     and transposes (d,t) -> (t,d) when assembling the full output
"""

import numpy as np
import ml_dtypes

B, T, E, H = 2, 2048, 1024, 16
D = 64          # head dim
HG = 4          # heads per core
GC = HG * D     # 256 output columns per core
NCORES = 8

_cached_nc = None

# tuning knobs (read at build time; defaults are the shipped config)
_P = {"sch_tail": 12, "stagger": True, "depth": 2,
      "tails2": False, "late1": 1, "late2": 2, "stagat": 2,
      "sring": 2}


def _build_program(seq: int = T, reps: int = 1, skip_attn=False, skip_exp=False, _nosplit=True):
    """reps>1 emits the body multiple times in one NEFF (timing only).
    skip_attn/skip_exp build ablation variants for HW phase attribution."""
    import concourse.bacc as bacc
    import concourse.tile as tile
    from concourse import mybir

    bf16 = mybir.dt.bfloat16
    f32 = mybir.dt.float32
    i16 = mybir.dt.int16
    Exp = mybir.ActivationFunctionType.Exp
    Mult = mybir.AluOpType.mult
    Add = mybir.AluOpType.add
    # Schraudolph fast-exp constants (2^x bit trick), with the 1/sqrt(HD)
    # score scale folded into the multiplier like the ACT path's scale=.
    # Scaled by 2^-16 so the rounded result IS the bf16 bit pattern as
    # int16 — one DVE op, no separate bitcast-narrow pass.
    SCH_C1 = (1 << 7) * 1.4426950408889634 * 0.125
    SCH_C2 = (1 << 7) * (127.0 - 0.04367744)
    SCH_TAIL = _P["sch_tail"]  # exp groups >= this index (per unit) -> DVE

    NT = seq // 128     # s-tiles / t-tiles
    NTC = seq // 512    # 512-wide t-chunks
    KO = E // 128       # contraction chunks for projections

    nc = bacc.Bacc(
        "TRN2", target_bir_lowering=False, debug=False, num_devices=NCORES
    )

    xT_d = nc.dram_tensor("xT", [E, seq], bf16, kind="ExternalInput")
    wq_d = nc.dram_tensor("wq", [E, GC], bf16, kind="ExternalInput")
    wk_d = nc.dram_tensor("wk", [E, GC], bf16, kind="ExternalInput")
    wv_d = nc.dram_tensor("wv", [E, GC], bf16, kind="ExternalInput")
    # un-normalized attn output, transposed: row block
    # ((p*NTC + tcq)*2 + h)*65 + d holds the unit's 512 t-columns for head
    # h of pair p; row 64 of each block is the softmax denominator. The
    # host divides and transposes when assembling the full output.
    out_d = nc.dram_tensor("out", [2 * NTC * 2 * 65, 512], f32,
                           kind="ExternalOutput")

    with tile.TileContext(nc) as tc:
        with (
            # per-stream pools: rep i+1's loads/projections are hoisted into
            # rep i's back half; program order already sequences the WAR
            # releases (each tensor's last use precedes its reload point on
            # the PE queue), so single buffering costs no stalls.
            tc.tile_pool(name="wqp", bufs=1) as wqp,
            tc.tile_pool(name="wkp", bufs=1) as wkp,
            tc.tile_pool(name="wvp", bufs=1) as wvp,
            tc.tile_pool(name="xtp", bufs=1) as xtp,
            tc.tile_pool(name="qtp", bufs=1) as qtp,
            tc.tile_pool(name="ktp", bufs=1) as ktp,
            tc.tile_pool(name="vp", bufs=1) as vp,
            tc.tile_pool(name="pt", bufs=3) as ptp,
            tc.tile_pool(name="stage", bufs=4) as stagep,
            # PSUM budget (8 banks): scores 2x[128,1024] (4) + attn
            # accumulators 3x[128,512] (3) + proj 1. The attn matmuls run
            # V-stationary at N=512, needing a bank per (unit, head); the
            # score ring gives up a slot to pay for it (exp keeps up — the
            # exp machine has slack behind the PE stream).
            tc.tile_pool(name="proj_ps", bufs=1, space="PSUM") as proj_ps,
            tc.tile_pool(name="score_ps", bufs=_P["sring"],
                         space="PSUM") as score_ps,
            tc.tile_pool(name="attn_ps", bufs=8 - 1 - 2 * _P["sring"],
                         space="PSUM") as attn_ps,
        ):
            # Weights ride the gpsimd DMA ring so their transfers overlap
            # the xT stream on the sync ring instead of queueing behind it.
            def dma_w(dst, src, p, ks=slice(0, KO)):
                cols = slice(p * 128, (p + 1) * 128)
                nc.gpsimd.dma_start(
                    dst[:, ks, cols],
                    src[:, cols].rearrange("(ko p) c -> p ko c", p=128)[:, ks],
                )

            def dma_x(xT, tcq, fine=False):
                sl = slice(tcq * 512, (tcq + 1) * 512)
                if fine:
                    # per-k-chunk pieces so the first projection chain's
                    # early matmuls start after a 128KB transfer, not 1MB.
                    for k in range(KO):
                        nc.sync.dma_start(
                            xT[:, k, sl], xT_d[k * 128:(k + 1) * 128, sl])
                else:
                    nc.sync.dma_start(
                        xT[:, :, sl],
                        xT_d[:, sl].rearrange("(ko p) c -> p ko c", p=128),
                    )

            def make_stream(fine):
                """Allocate one rep's input/projection tiles and issue the
                loads, ramp-ordered: the first projection chains need only
                the pair-0 weights + xT t-chunk 0 (DMA-bandwidth-bound, so
                those transfers lead on their rings)."""
                S = {
                    "wq": wqp.tile([128, KO, GC], bf16, name="wq"),
                    "wk": wkp.tile([128, KO, GC], bf16, name="wk"),
                    "wv": wvp.tile([128, KO, GC], bf16, name="wv"),
                    "xT": xtp.tile([128, KO, seq], bf16, name="xT"),
                    "QT": qtp.tile([128, 2, seq], bf16, name="QT"),
                    "KT": ktp.tile([128, 2, seq], bf16, name="KT"),
                    "V": vp.tile([128, NT, HG * (D + 1)], bf16, name="V"),
                }
                dma_w(S["wq"], wq_d, 0)
                dma_w(S["wk"], wk_d, 0)
                dma_x(S["xT"], 0, fine=fine)
                dma_w(S["wq"], wq_d, 1)
                dma_w(S["wk"], wk_d, 1)
                for tcq in range(1, NTC):
                    dma_x(S["xT"], tcq)
                nc.gpsimd.dma_start(
                    S["wv"][:],
                    wv_d[:].rearrange("(ko p) c -> p ko c", p=128))
                # ones column per head; proj_v overwrites the data columns
                nc.vector.memset(
                    S["V"][:].rearrange(
                        "p st (h c) -> p st h c", h=HG)[:, :, :, D:D + 1],
                    1.0,
                )
                return S

            def proj_qk(S, p, w_sb, dst, tcs):
                """Project t-chunks `tcs` of QT or KT for head-pair p."""
                for tcq in tcs:
                    ps = proj_ps.tile([128, 512], f32, tag="proj")
                    for k in range(KO):
                        nc.tensor.matmul(
                            ps[:],
                            lhsT=w_sb[:, k, p * 128:(p + 1) * 128],
                            rhs=S["xT"][:, k, tcq * 512:(tcq + 1) * 512],
                            start=(k == 0),
                            stop=(k == KO - 1),
                        )
                    nc.vector.tensor_copy(
                        out=dst[:, p, tcq * 512:(tcq + 1) * 512], in_=ps[:]
                    )

            def proj_v(S, tiles):
                for st in tiles:
                    ps = proj_ps.tile([128, 512], f32, tag="proj")
                    for k in range(KO):
                        nc.tensor.matmul(
                            ps[:, :GC],
                            lhsT=S["xT"][:, k, st * 128:(st + 1) * 128],
                            rhs=S["wv"][:, k, :],
                            start=(k == 0),
                            stop=(k == KO - 1),
                        )
                    nc.vector.tensor_copy(
                        out=S["V"][:, st].rearrange(
                            "p (h c) -> p h c", h=HG)[:, :, :D],
                        in_=ps[:, :GC].rearrange("p (h c) -> p h c", h=HG),
                    )

            def leading_proj(S):
                proj_qk(S, 0, S["wq"], S["QT"], [0])
                proj_qk(S, 0, S["wk"], S["KT"], [0])

            def scores_unit(S, p, t0, hooks=None, n_sch=None, tail=False,
                            w=512):
                """ST = K Q^T (both heads row-packed) + exp -> PT tile.

                PT layout is flat [128, NT*1024]: one s-tile's two 512-wide
                bank writes form one 2-bank exp group, so attn can chase exp
                output at s-tile granularity. n_sch groups take the DVE
                Schraudolph path: a single tensor_scalar whose rounded int16
                result is the bf16 bit pattern of 2^x (ACT alone can't keep
                up with PE in units that carry no projection filler). The
                last unit alternates engines per group so its exps drain at
                the combined ACT+DVE rate. hooks[st] emits filler just
                before s-tile st."""
                # constant tile shapes regardless of w — half-width units
                # just use the leading columns
                pt = ptp.tile([128, NT * 1024], bf16, tag="pt")
                if skip_exp:
                    # ablation builds: allocate pt so downstream reads are
                    # legal even though no exp writes it
                    nc.vector.memset(pt[:, :1], 1.0)
                if n_sch is None:
                    n_sch = NT - SCH_TAIL
                # Spread DVE groups evenly through the unit (clustering them
                # at the tail leaves DVE idle while ACT backlogs, and the
                # score PSUM ring is only 3 groups deep).
                sch_set = {st for st in range(NT)
                           if ((st + 1) * n_sch) // NT > (st * n_sch) // NT}
                for st in range(NT):
                    for f in (hooks or {}).get(st, []):
                        f()
                    sc = score_ps.tile([128, 1024], f32, tag="score")
                    for h in range(2):
                        # heads stay bank-strided (h*512, not h*w): the two
                        # matmuls run concurrently on different PE row
                        # groups, and concurrent writes into one single-port
                        # PSUM bank are a hardware fault.
                        nc.tensor.matmul(
                            sc[:, h * 512:h * 512 + w],
                            lhsT=S["KT"][h * 64:(h + 1) * 64, p,
                                         st * 128:(st + 1) * 128],
                            rhs=S["QT"][h * 64:(h + 1) * 64, p, t0:t0 + w],
                            start=True,
                            stop=True,
                        )
                    if skip_exp:
                        continue
                    dst = pt[:, st * 2 * w:(st + 1) * 2 * w]
                    src = (sc[:] if w == 512 else
                           sc[:].rearrange("p (b c) -> p b c", b=2)[:, :, :w])
                    if (st % 2 == 1) if tail else (st in sch_set):
                        nc.vector.tensor_scalar(
                            dst.bitcast(i16), src, SCH_C1, SCH_C2,
                            Mult, Add,
                        )
                    else:
                        nc.scalar.activation(
                            out=dst, in_=src, func=Exp, scale=0.125,
                        )
                return pt

            def attn_unit(S, p, t0, pt, w=512, fine_out=False):
                """acc[d, t] = V_aug^T @ PT, V-stationary at N=w: one
                65-column weight load per (head, s-tile) feeds a full
                512-wide stream, so the per-matmul LDWEIGHTS+dispatch tax is
                paid 32x per unit instead of 128x. Row 64 accumulates the
                softmax denominator via V's ones column. Each head gets its
                own PSUM bank; the acc is copied to SBUF and DMAd out
                un-normalized (the host divides and transposes)."""
                tcq = t0 // 512
                toff = t0 % 512
                for h in range(2):
                    hh = p * 2 + h
                    acc = attn_ps.tile([128, 512], f32, tag="attn")
                    for st in range(NT):
                        nc.tensor.matmul(
                            acc[:D + 1, :w],
                            lhsT=S["V"][:, st,
                                        hh * (D + 1):(hh + 1) * (D + 1)],
                            rhs=pt[:, st * 2 * w + h * w:
                                   st * 2 * w + (h + 1) * w],
                            start=(st == 0),
                            stop=(st == NT - 1),
                        )
                    stg = stagep.tile([128, 512], f32, tag="stage")
                    nc.vector.tensor_copy(out=stg[:D + 1, :w],
                                          in_=acc[:D + 1, :w])
                    base = ((p * NTC + tcq) * 2 + h) * 65
                    nc.sync.dma_start(
                        out_d[base:base + 65, toff:toff + w],
                        stg[:D + 1, :w])

            # Program order is semantic order under Tile (WAR/RAW follow it),
            # and it is also the scheduler's priority order. Software-pipeline
            # the softmax: emit scores(u+1) before attn(u) so ACT never
            # starves at a unit boundary; slot filler work (V projection,
            # pair-1 QK, deferred QT-0 chunks) right after the scores that
            # precede its first use.
            # Minimal critical path to the first exp: QT0[tc0], KT0[tc0],
            # then unit-0 scores. All remaining projection work (KT0 tails,
            # QT0 tails, V, pair-1 QK) is spread through the score s-loops
            # as hook filler so PE keeps ACT fed instead of lumping
            # projections between units. attn runs two units behind scores.
            # Everything is emitted before its first program-order use.
            # Across reps: the next rep's loads are hoisted into unit 5 and
            # its leading projections run between the last scores unit and
            # the attn drain, so the PE never idles at a rep boundary.
            def body(S, last):
                S2 = [None]
                qk0 = lambda w_, d_, tcs: (
                    lambda: proj_qk(S, 0, S[w_], S[d_], tcs))
                qk1 = lambda w_, d_, tcs: (
                    lambda: proj_qk(S, 1, S[w_], S[d_], tcs))
                pv = lambda ts: (lambda: proj_v(S, ts))
                if NTC == 4:
                    # Each chunk is hooked 2-4 s-tiles before its first use
                    # so the PSUM->SBUF copy lands before the dependent
                    # ld/mm instead of just-in-time (the copy latency
                    # otherwise stalls the score pipeline at every chunk
                    # boundary).
                    hooks = {
                        0: {2: [qk0("wk", "KT", [1])], 5: [qk0("wk", "KT", [2])],
                            8: [qk0("wk", "KT", [3])], 12: [qk0("wq", "QT", [1])]},
                        1: {2: [pv(range(0, 4))], 6: [qk0("wq", "QT", [2])],
                            10: [pv(range(4, 8))], 14: [qk0("wq", "QT", [3])]},
                        2: {2: [pv(range(8, 12))], 5: [qk1("wk", "KT", [0])],
                            8: [pv(range(12, 16))], 11: [qk1("wk", "KT", [1])],
                            14: [qk1("wk", "KT", [2])]},
                        3: {2: [qk1("wk", "KT", [3])], 6: [qk1("wq", "QT", [0])],
                            10: [qk1("wq", "QT", [1])], 14: [qk1("wq", "QT", [2])]},
                        4: {2: [qk1("wq", "QT", [3])]},
                    }
                    fillers = {}
                else:
                    hooks = {0: {4 * c: [qk0("wk", "KT", [c])]
                                 for c in range(1, NTC)}}
                    fillers = {0: [qk0("wq", "QT", range(1, NTC)),
                                   pv(range(NT))]}
                    fillers.setdefault(min(1, NTC - 1), []).extend((
                        qk1("wk", "KT", range(NTC)),))
                    fillers.setdefault(min(2, NTC - 1), []).append(
                        qk1("wq", "QT", range(NTC)))
                units = [(p, tcq * 512, 512)
                         for p in range(2) for tcq in range(NTC)]
                # The final unit is split into two 256-wide halves so the
                # last exp batch and attn chains are half-sized — the
                # pipeline drain after the last score matmul shortens
                # accordingly.
                if NTC == 4 and not _nosplit:
                    lp, lt, _ = units.pop()
                    units += [(lp, lt, 256), (lp, lt + 256, 256)]
                    n_sch = {5: NT - SCH_TAIL + 1, 6: NT - SCH_TAIL + 2}
                    tails = {7, 8}
                else:
                    n_sch = {len(units) - 3: NT - SCH_TAIL + _P["late1"],
                             len(units) - 2: NT - SCH_TAIL + _P["late2"]}
                    tails = ({len(units) - 2, len(units) - 1} if _P["tails2"]
                             else {len(units) - 1})
                nu = len(units)
                if not last:
                    # hoist the next rep's loads: emitted mid-body so the
                    # transfers run behind this rep's back half, and the
                    # leading projections fill the PE between the last
                    # scores unit and the attn drain.
                    def mk_next():
                        S2[0] = make_stream(fine=False)
                    hooks.setdefault(min(5, nu - 3), {}).setdefault(
                        8, []).insert(0, mk_next)
                    fillers.setdefault(nu - 1, []).append(
                        lambda: leading_proj(S2[0]))
                # Units with hook filler keep PE busy past ACT's exp rate;
                # the late filler-free units need a bigger DVE share, and
                # the tail units drain on both engines at once.
                pending = []  # [(p, t0, pt, w)] awaiting attn
                for i, (p, t0, w) in enumerate(units):
                    pt = scores_unit(S, p, t0, hooks.get(i), n_sch.get(i),
                                     tail=(i in tails), w=w)
                    for f in fillers.get(i, []):
                        f()
                    # attn lags scores by 2 units mid-stream (pt/exp slack)
                    # but drops to 1 near the end, so only a single attn
                    # unit remains after the final score matmul.
                    depth = (_P["depth"]
                             if (i < nu - _P["stagat"] or not _P["stagger"])
                             else 1)
                    while len(pending) >= depth:
                        if not skip_attn:
                            attn_unit(S, *pending.pop(0))
                        else:
                            pending.pop(0)
                    pending.append((p, t0, pt, w))
                for j, args in enumerate(pending):
                    if not skip_attn:
                        attn_unit(S, *args, fine_out=(j == len(pending) - 1))
                return S2[0]

            S = make_stream(fine=True)
            leading_proj(S)
            for _rep in range(reps):
                S = body(S, last=(_rep == reps - 1))

    nc.compile()
    return nc


def _shard_inputs(x, w_Q, w_K, w_V):
    bf = ml_dtypes.bfloat16
    in_maps = []
    for c in range(NCORES):
        b, g = divmod(c, NCORES // B)
        cols = slice(g * GC, (g + 1) * GC)
        in_maps.append({
            "xT": np.ascontiguousarray(np.asarray(x)[b].T).astype(bf),
            "wq": np.ascontiguousarray(np.asarray(w_Q)[:, cols]).astype(bf),
            "wk": np.ascontiguousarray(np.asarray(w_K)[:, cols]).astype(bf),
            "wv": np.ascontiguousarray(np.asarray(w_V)[:, cols]).astype(bf),
        })
    return in_maps


def kernel(x, w_Q, w_K, w_V, _trace=False, _tmpdir=None):
    from concourse.bass_utils import run_bass_kernel_spmd

    global _cached_nc
    if _cached_nc is None:
        _cached_nc = _build_program(T)
    in_maps = _shard_inputs(x, w_Q, w_K, w_V)
    res = run_bass_kernel_spmd(
        _cached_nc, in_maps, list(range(NCORES)),
        trace=_trace, tmpdir=_tmpdir,
    )
    NTC = T // 512
    out = np.empty((B, T, E), np.float32)
    for c in range(NCORES):
        b, g = divmod(c, NCORES // B)
        # blocks [p, tcq, h, d(65), tc]: row 64 is the denominator;
        # t = tcq*512 + tc, col = g*GC + p*128 + h*64 + d
        arr = res.results[c]["out"].reshape(2, NTC, 2, 65, 512)
        o = arr[:, :, :, :64] / arr[:, :, :, 64:65]
        o = o.transpose(1, 4, 0, 2, 3).reshape(T, GC)
        out[b, :, g * GC:(g + 1) * GC] = o
    if _trace:
        return out, res
    return out



# revision 42
# speedup vs baseline: 1.3755x; 1.3755x over previous
"""Multi-head attention (B=2, T=2048, E=1024, H=16) on 8 TRN2 NeuronCores.

Sharding: core c handles batch c//4 and head group c%4 (4 heads of 64 dims
-> 256 columns of w_Q/w_K/w_V and of the output). Pure SPMD, no collectives:
every core runs the same NEFF on its own input shard.

Per-core kernel (all matmul operands bf16, PSUM/softmax math fp32):
  xT [E, T] (host pre-transposed), wq/wk/wv [E, 256]
  1. QT/KT per head-pair p: [128, T] = (w pair-slice)^T @ xT   (PE)
  2. V per s-tile: [128, 4*65] with a ones column per head     (PE + DVE copy)
  3. scores transposed per head: ST[s, t] = K Q^T, two heads packed into
     PE row groups (K=64 each) writing one [128, 1024] PSUM tile
  4. exp via ACT straight from PSUM, scale=1/8 folded into the activation
     affine, bf16 out -> PT
  5. attn: out[t,65] = PT_slice^T @ V_aug accumulated over 16 s-chunks;
     col 64 = softmax denominator (from the ones column)
  6. normalize: DVE reciprocal + per-partition tensor_scalar mul -> fp32 out
"""

import numpy as np
import ml_dtypes

B, T, E, H = 2, 2048, 1024, 16
D = 64          # head dim
HG = 4          # heads per core
GC = HG * D     # 256 output columns per core
NCORES = 8

_cached_nc = None

# tuning knobs (read at build time; defaults are the shipped config)
_P = {"sch_tail": 12, "stagger": True, "depth": 2,
      "tails2": False, "late1": 1, "late2": 2, "stagat": 2,
      "groups3": False}


def _build_program(seq: int = T, reps: int = 1, skip_attn=False, skip_exp=False, _nosplit=True):
    """reps>1 emits the body multiple times in one NEFF (timing only).
    skip_attn/skip_exp build ablation variants for HW phase attribution."""
    import concourse.bacc as bacc
    import concourse.tile as tile
    from concourse import mybir

    bf16 = mybir.dt.bfloat16
    f32 = mybir.dt.float32
    i16 = mybir.dt.int16
    Exp = mybir.ActivationFunctionType.Exp
    Mult = mybir.AluOpType.mult
    Add = mybir.AluOpType.add
    # Schraudolph fast-exp constants (2^x bit trick), with the 1/sqrt(HD)
    # score scale folded into the multiplier like the ACT path's scale=.
    # Scaled by 2^-16 so the rounded result IS the bf16 bit pattern as
    # int16 — one DVE op, no separate bitcast-narrow pass.
    SCH_C1 = (1 << 7) * 1.4426950408889634 * 0.125
    SCH_C2 = (1 << 7) * (127.0 - 0.04367744)
    SCH_TAIL = _P["sch_tail"]  # exp groups >= this index (per unit) -> DVE

    NT = seq // 128     # s-tiles / t-tiles
    NTC = seq // 512    # 512-wide t-chunks
    KO = E // 128       # contraction chunks for projections

    nc = bacc.Bacc(
        "TRN2", target_bir_lowering=False, debug=False, num_devices=NCORES
    )

    xT_d = nc.dram_tensor("xT", [E, seq], bf16, kind="ExternalInput")
    wq_d = nc.dram_tensor("wq", [E, GC], bf16, kind="ExternalInput")
    wk_d = nc.dram_tensor("wk", [E, GC], bf16, kind="ExternalInput")
    wv_d = nc.dram_tensor("wv", [E, GC], bf16, kind="ExternalInput")
    out_d = nc.dram_tensor("out", [seq, GC], f32, kind="ExternalOutput")

    with tile.TileContext(nc) as tc:
        with (
            # per-stream pools: rep i+1's loads/projections are hoisted into
            # rep i's back half; program order already sequences the WAR
            # releases (each tensor's last use precedes its reload point on
            # the PE queue), so single buffering costs no stalls.
            tc.tile_pool(name="wqp", bufs=1) as wqp,
            tc.tile_pool(name="wkp", bufs=1) as wkp,
            tc.tile_pool(name="wvp", bufs=1) as wvp,
            tc.tile_pool(name="xtp", bufs=1) as xtp,
            tc.tile_pool(name="qtp", bufs=1) as qtp,
            tc.tile_pool(name="ktp", bufs=1) as ktp,
            tc.tile_pool(name="vp", bufs=1) as vp,
            tc.tile_pool(name="pt", bufs=3) as ptp,
            tc.tile_pool(name="stage", bufs=8) as stagep,
            tc.tile_pool(name="recip", bufs=8) as recipp,
            # PSUM budget (8 banks): scores 3x[128,1024] (6) + attn 1 + proj 1
            tc.tile_pool(name="proj_ps", bufs=1, space="PSUM") as proj_ps,
            tc.tile_pool(name="score_ps", bufs=3, space="PSUM") as score_ps,
            tc.tile_pool(name="attn_ps", bufs=1, space="PSUM") as attn_ps,
        ):
            # Weights ride the gpsimd DMA ring so their transfers overlap
            # the xT stream on the sync ring instead of queueing behind it.
            def dma_w(dst, src, p, ks=slice(0, KO)):
                cols = slice(p * 128, (p + 1) * 128)
                nc.gpsimd.dma_start(
                    dst[:, ks, cols],
                    src[:, cols].rearrange("(ko p) c -> p ko c", p=128)[:, ks],
                )

            def dma_x(xT, tcq, fine=False):
                sl = slice(tcq * 512, (tcq + 1) * 512)
                if fine:
                    # per-k-chunk pieces so the first projection chain's
                    # early matmuls start after a 128KB transfer, not 1MB.
                    for k in range(KO):
                        nc.sync.dma_start(
                            xT[:, k, sl], xT_d[k * 128:(k + 1) * 128, sl])
                else:
                    nc.sync.dma_start(
                        xT[:, :, sl],
                        xT_d[:, sl].rearrange("(ko p) c -> p ko c", p=128),
                    )

            def make_stream(fine):
                """Allocate one rep's input/projection tiles and issue the
                loads, ramp-ordered: the first projection chains need only
                the pair-0 weights + xT t-chunk 0 (DMA-bandwidth-bound, so
                those transfers lead on their rings)."""
                S = {
                    "wq": wqp.tile([128, KO, GC], bf16, name="wq"),
                    "wk": wkp.tile([128, KO, GC], bf16, name="wk"),
                    "wv": wvp.tile([128, KO, GC], bf16, name="wv"),
                    "xT": xtp.tile([128, KO, seq], bf16, name="xT"),
                    "QT": qtp.tile([128, 2, seq], bf16, name="QT"),
                    "KT": ktp.tile([128, 2, seq], bf16, name="KT"),
                    "V": vp.tile([128, NT, HG * (D + 1)], bf16, name="V"),
                }
                dma_w(S["wq"], wq_d, 0)
                dma_w(S["wk"], wk_d, 0)
                dma_x(S["xT"], 0, fine=fine)
                dma_w(S["wq"], wq_d, 1)
                dma_w(S["wk"], wk_d, 1)
                for tcq in range(1, NTC):
                    dma_x(S["xT"], tcq)
                nc.gpsimd.dma_start(
                    S["wv"][:],
                    wv_d[:].rearrange("(ko p) c -> p ko c", p=128))
                # ones column per head; proj_v overwrites the data columns
                nc.vector.memset(
                    S["V"][:].rearrange(
                        "p st (h c) -> p st h c", h=HG)[:, :, :, D:D + 1],
                    1.0,
                )
                return S

            def proj_qk(S, p, w_sb, dst, tcs):
                """Project t-chunks `tcs` of QT or KT for head-pair p."""
                for tcq in tcs:
                    ps = proj_ps.tile([128, 512], f32, tag="proj")
                    for k in range(KO):
                        nc.tensor.matmul(
                            ps[:],
                            lhsT=w_sb[:, k, p * 128:(p + 1) * 128],
                            rhs=S["xT"][:, k, tcq * 512:(tcq + 1) * 512],
                            start=(k == 0),
                            stop=(k == KO - 1),
                        )
                    nc.vector.tensor_copy(
                        out=dst[:, p, tcq * 512:(tcq + 1) * 512], in_=ps[:]
                    )

            def proj_v(S, tiles):
                for st in tiles:
                    ps = proj_ps.tile([128, 512], f32, tag="proj")
                    for k in range(KO):
                        nc.tensor.matmul(
                            ps[:, :GC],
                            lhsT=S["xT"][:, k, st * 128:(st + 1) * 128],
                            rhs=S["wv"][:, k, :],
                            start=(k == 0),
                            stop=(k == KO - 1),
                        )
                    nc.vector.tensor_copy(
                        out=S["V"][:, st].rearrange(
                            "p (h c) -> p h c", h=HG)[:, :, :D],
                        in_=ps[:, :GC].rearrange("p (h c) -> p h c", h=HG),
                    )

            def leading_proj(S):
                proj_qk(S, 0, S["wq"], S["QT"], [0])
                proj_qk(S, 0, S["wk"], S["KT"], [0])

            def scores_unit(S, p, t0, hooks=None, n_sch=None, tail=False,
                            w=512):
                """ST = K Q^T (both heads row-packed) + exp -> PT tile.

                PT layout is flat [128, NT*1024]: one s-tile's two 512-wide
                bank writes form one 2-bank exp group, so attn can chase exp
                output at s-tile granularity. n_sch groups take the DVE
                Schraudolph path: a single tensor_scalar whose rounded int16
                result is the bf16 bit pattern of 2^x (ACT alone can't keep
                up with PE in units that carry no projection filler). The
                last unit alternates engines per group so its exps drain at
                the combined ACT+DVE rate. hooks[st] emits filler just
                before s-tile st."""
                # constant tile shapes regardless of w — half-width units
                # just use the leading columns
                pt = ptp.tile([128, NT * 1024], bf16, tag="pt")
                if skip_exp:
                    # ablation builds: allocate pt so downstream reads are
                    # legal even though no exp writes it
                    nc.vector.memset(pt[:, :1], 1.0)
                if n_sch is None:
                    n_sch = NT - SCH_TAIL
                # Spread DVE groups evenly through the unit (clustering them
                # at the tail leaves DVE idle while ACT backlogs, and the
                # score PSUM ring is only 3 groups deep).
                sch_set = {st for st in range(NT)
                           if ((st + 1) * n_sch) // NT > (st * n_sch) // NT}
                for st in range(NT):
                    for f in (hooks or {}).get(st, []):
                        f()
                    sc = score_ps.tile([128, 1024], f32, tag="score")
                    for h in range(2):
                        # heads stay bank-strided (h*512, not h*w): the two
                        # matmuls run concurrently on different PE row
                        # groups, and concurrent writes into one single-port
                        # PSUM bank are a hardware fault.
                        nc.tensor.matmul(
                            sc[:, h * 512:h * 512 + w],
                            lhsT=S["KT"][h * 64:(h + 1) * 64, p,
                                         st * 128:(st + 1) * 128],
                            rhs=S["QT"][h * 64:(h + 1) * 64, p, t0:t0 + w],
                            start=True,
                            stop=True,
                        )
                    if skip_exp:
                        continue
                    dst = pt[:, st * 2 * w:(st + 1) * 2 * w]
                    src = (sc[:] if w == 512 else
                           sc[:].rearrange("p (b c) -> p b c", b=2)[:, :, :w])
                    if (st % 2 == 1) if tail else (st in sch_set):
                        nc.vector.tensor_scalar(
                            dst.bitcast(i16), src, SCH_C1, SCH_C2,
                            Mult, Add,
                        )
                    else:
                        nc.scalar.activation(
                            out=dst, in_=src, func=Exp, scale=0.125,
                        )
                return pt

            def attn_unit(S, p, t0, pt, w=512, fine_out=False):
                """attn = PT^T @ V_aug accumulated over s, then normalize.
                tt-major: both heads' accumulators for one t-tile share a
                PSUM bank (each chain is a closed start..stop group), so
                each t-tile normalizes and DMAs out while the next t-tile's
                chains run — the unit's output drains incrementally instead
                of all at the end."""
                ap2 = attn_ps.tile([128, 2, 2 * (D + 1)], f32, tag="attn")
                stg2 = None
                for tt in range(w // 128):
                    # alternate halves of the bank: t-tile tt+2's first
                    # (start=True) write only has to wait for tt's normalize
                    # reads, two chains back — not the immediately preceding
                    # ones.
                    ap = ap2[:, tt % 2]
                    if stg2 is None:
                        stg2 = stagep.tile([128, 2, 128], f32, tag="stage")
                    for h in range(2):
                        hh = p * 2 + h
                        for st in range(NT):
                            nc.tensor.matmul(
                                ap[:, h * (D + 1):(h + 1) * (D + 1)],
                                lhsT=pt[:, st * 2 * w + h * w + tt * 128:
                                        st * 2 * w + h * w + (tt + 1) * 128],
                                rhs=S["V"][:, st,
                                           hh * (D + 1):(hh + 1) * (D + 1)],
                                start=(st == 0),
                                stop=(st == NT - 1),
                            )
                    for h in range(2):
                        r = recipp.tile([128, 1], f32, tag="recip")
                        nc.vector.reciprocal(
                            out=r[:],
                            in_=ap[:, h * (D + 1) + D:h * (D + 1) + D + 1],
                        )
                        nc.vector.tensor_scalar(
                            stg2[:, tt % 2, h * D:(h + 1) * D],
                            ap[:, h * (D + 1):h * (D + 1) + D],
                            r[:],
                            None,
                            Mult,
                        )
                    # one out-DMA per tt-pair: fewer DGE setups in flight
                    # than per-tt writes, at one extra normalize of latency.
                    # The final unit keeps per-tt writes so the kernel's
                    # closing DMA starts as early as possible.
                    if fine_out or tt % 2 == 1 or tt == w // 128 - 1:
                        n = 1 if fine_out else tt % 2 + 1
                        tb = tt - n + 1
                        base = tt % 2 - n + 1
                        nc.sync.dma_start(
                            out_d[t0 + tb * 128:t0 + (tt + 1) * 128,
                                  p * 128:(p + 1) * 128]
                            .rearrange("(j q) c -> q j c", q=128),
                            stg2[:, base:base + n],
                        )
                        if tt % 2 == 1:
                            stg2 = None

            # Program order is semantic order under Tile (WAR/RAW follow it),
            # and it is also the scheduler's priority order. Software-pipeline
            # the softmax: emit scores(u+1) before attn(u) so ACT never
            # starves at a unit boundary; slot filler work (V projection,
            # pair-1 QK, deferred QT-0 chunks) right after the scores that
            # precede its first use.
            # Minimal critical path to the first exp: QT0[tc0], KT0[tc0],
            # then unit-0 scores. All remaining projection work (KT0 tails,
            # QT0 tails, V, pair-1 QK) is spread through the score s-loops
            # as hook filler so PE keeps ACT fed instead of lumping
            # projections between units. attn runs two units behind scores.
            # Everything is emitted before its first program-order use.
            # Across reps: the next rep's loads are hoisted into unit 5 and
            # its leading projections run between the last scores unit and
            # the attn drain, so the PE never idles at a rep boundary.
            def body(S, last):
                S2 = [None]
                qk0 = lambda w_, d_, tcs: (
                    lambda: proj_qk(S, 0, S[w_], S[d_], tcs))
                qk1 = lambda w_, d_, tcs: (
                    lambda: proj_qk(S, 1, S[w_], S[d_], tcs))
                pv = lambda ts: (lambda: proj_v(S, ts))
                if NTC == 4:
                    # Each chunk is hooked 2-4 s-tiles before its first use
                    # so the PSUM->SBUF copy lands before the dependent
                    # ld/mm instead of just-in-time (the copy latency
                    # otherwise stalls the score pipeline at every chunk
                    # boundary).
                    hooks = {
                        0: {2: [qk0("wk", "KT", [1])], 5: [qk0("wk", "KT", [2])],
                            8: [qk0("wk", "KT", [3])], 12: [qk0("wq", "QT", [1])]},
                        1: {2: [pv(range(0, 4))], 6: [qk0("wq", "QT", [2])],
                            10: [pv(range(4, 8))], 14: [qk0("wq", "QT", [3])]},
                        2: {2: [pv(range(8, 12))], 5: [qk1("wk", "KT", [0])],
                            8: [pv(range(12, 16))], 11: [qk1("wk", "KT", [1])],
                            14: [qk1("wk", "KT", [2])]},
                        3: {2: [qk1("wk", "KT", [3])], 6: [qk1("wq", "QT", [0])],
                            10: [qk1("wq", "QT", [1])], 14: [qk1("wq", "QT", [2])]},
                        4: {2: [qk1("wq", "QT", [3])]},
                    }
                    fillers = {}
                else:
                    hooks = {0: {4 * c: [qk0("wk", "KT", [c])]
                                 for c in range(1, NTC)}}
                    fillers = {0: [qk0("wq", "QT", range(1, NTC)),
                                   pv(range(NT))]}
                    fillers.setdefault(min(1, NTC - 1), []).extend((
                        qk1("wk", "KT", range(NTC)),))
                    fillers.setdefault(min(2, NTC - 1), []).append(
                        qk1("wq", "QT", range(NTC)))
                units = [(p, tcq * 512, 512)
                         for p in range(2) for tcq in range(NTC)]
                # The final unit is split into two 256-wide halves so the
                # last exp batch and attn chains are half-sized — the
                # pipeline drain after the last score matmul shortens
                # accordingly.
                if NTC == 4 and not _nosplit:
                    lp, lt, _ = units.pop()
                    units += [(lp, lt, 256), (lp, lt + 256, 256)]
                    n_sch = {5: NT - SCH_TAIL + 1, 6: NT - SCH_TAIL + 2}
                    tails = {7, 8}
                else:
                    n_sch = {len(units) - 3: NT - SCH_TAIL + _P["late1"],
                             len(units) - 2: NT - SCH_TAIL + _P["late2"]}
                    tails = ({len(units) - 2, len(units) - 1} if _P["tails2"]
                             else {len(units) - 1})
                nu = len(units)
                if not last:
                    # hoist the next rep's loads: emitted mid-body so the
                    # transfers run behind this rep's back half, and the
                    # leading projections fill the PE between the last
                    # scores unit and the attn drain.
                    def mk_next():
                        S2[0] = make_stream(fine=False)
                    hooks.setdefault(min(5, nu - 3), {}).setdefault(
                        8, []).insert(0, mk_next)
                    fillers.setdefault(nu - 1, []).append(
                        lambda: leading_proj(S2[0]))
                # Units with hook filler keep PE busy past ACT's exp rate;
                # the late filler-free units need a bigger DVE share, and
                # the tail units drain on both engines at once.
                pending = []  # [(p, t0, pt, w)] awaiting attn
                for i, (p, t0, w) in enumerate(units):
                    pt = scores_unit(S, p, t0, hooks.get(i), n_sch.get(i),
                                     tail=(i in tails), w=w)
                    for f in fillers.get(i, []):
                        f()
                    # attn lags scores by 2 units mid-stream (pt/exp slack)
                    # but drops to 1 near the end, so only a single attn
                    # unit remains after the final score matmul.
                    depth = (_P["depth"]
                             if (i < nu - _P["stagat"] or not _P["stagger"])
                             else 1)
                    while len(pending) >= depth:
                        if not skip_attn:
                            attn_unit(S, *pending.pop(0))
                        else:
                            pending.pop(0)
                    pending.append((p, t0, pt, w))
                for j, args in enumerate(pending):
                    if not skip_attn:
                        attn_unit(S, *args, fine_out=(j == len(pending) - 1))
                return S2[0]

            S = make_stream(fine=True)
            leading_proj(S)
            for _rep in range(reps):
                S = body(S, last=(_rep == reps - 1))

    nc.compile()
    return nc


def _shard_inputs(x, w_Q, w_K, w_V):
    bf = ml_dtypes.bfloat16
    in_maps = []
    for c in range(NCORES):
        b, g = divmod(c, NCORES // B)
        cols = slice(g * GC, (g + 1) * GC)
        in_maps.append({
            "xT": np.ascontiguousarray(np.asarray(x)[b].T).astype(bf),
            "wq": np.ascontiguousarray(np.asarray(w_Q)[:, cols]).astype(bf),
            "wk": np.ascontiguousarray(np.asarray(w_K)[:, cols]).astype(bf),
            "wv": np.ascontiguousarray(np.asarray(w_V)[:, cols]).astype(bf),
        })
    return in_maps


def kernel(x, w_Q, w_K, w_V, _trace=False, _tmpdir=None):
    from concourse.bass_utils import run_bass_kernel_spmd

    global _cached_nc
    if _cached_nc is None:
        _cached_nc = _build_program(T)
    in_maps = _shard_inputs(x, w_Q, w_K, w_V)
    res = run_bass_kernel_spmd(
        _cached_nc, in_maps, list(range(NCORES)),
        trace=_trace, tmpdir=_tmpdir,
    )
    out = np.empty((B, T, E), np.float32)
    for c in range(NCORES):
        b, g = divmod(c, NCORES // B)
        out[b, :, g * GC:(g + 1) * GC] = res.results[c]["out"]
    if _trace:
        return out, res
    return out

